# revision 43
# baseline (speedup 1.0000x reference)
"""Trainium2 Bass kernel for the two-level Haar-DWT detail (L1) loss.

Strategy (pure data parallel over batch, 8 NeuronCores):
  - Each core gets 4 of the 32 batch images (both `output` and `target`),
    viewed as a [6144, 512] row matrix; 24 pair-tiles of [128, 2, 512].
  - HBM roofline: each core must read 24 MiB at ~358 GB/s -> ~70 us.
    Everything else is kept below the ~2.7 us/pair stream rate:
      * o loads on the SP HWDGE ring as f32; t loads via GpSimd SWDGE
        with an in-DMA f32->bf16 cast.  Neither issuing sequencer runs
        sem-gated compute, so DMA issue never waits on the compute
        chain; the first two pairs load as 256 KiB halves and the
        weights ride the ACT HWDGE ring so pair 0 lands early.
      * The host pre-deinterleaves image columns mod 4 (new col
        n*128 + c = orig col 4c + n), which turns every DWT column
        pair-combine into a contiguous 128-wide block op.
      * The TensorEngine does the subtract AND the level-1 row combines
        in one pass: per 512-col block, a float32r matmul on o (full
        rate at N=512, needs no conversion pass) accumulates with a
        bf16 matmul on the negated weights applied to the bf16 t tile,
        giving S = Q*rowsum(o-t), D = rowdiff(o-t) in single-bank PSUM
        tiles (Q = 0.125, exact in bf16/f32r).
      * The PSUM->SBUF bf16 cast is split column-wise between ACT and
        DVE (engine cost is free-dim-driven); DVE then does the level-1
        and level-2 column combines as contiguous bf16 2x block ops.
  - ABS + accumulate runs on ACT per 4-pair group ([128, 2048] calls,
    and the level-2 bands of 4 pairs stack into all 128 PSUM
    partitions); each group's three ABS calls are deferred and drained
    one per later pair-slot so they never block the casts; the last
    group runs per-pair so the post-loop tail stays short.
  - The regions needing loss weight 0.1 (LL1) or 1 (HL1) or the 0.5
    level weight are re-scaled per partition range in the host combine,
    since ACT's accum_out is per-partition.
  - Each core emits [128, 8]; host combines in float64.
"""

import numpy as np

B, C, H, W = 32, 3, 512, 512
N_CORES = 8
B_PER_CORE = B // N_CORES
ROWS = B_PER_CORE * C * H  # 6144
COLS = W  # 512
NP = ROWS // 256  # 24 tile-pairs of [128, 2, 512]
Q = 0.125  # S-path scale: exact in bf16/f32r; host maps to the 0.1 LL weight

_CACHE = {}


def _make_weights():
    import ml_dtypes
    # wo[k, m] (f32): row combines for the o operand; wt = -wo for t.
    # m<64: +Q at rows 2m, 2m+1 (pair sum  -> S = Q*rowsum(o-t));
    # m=64+mm: -1/+1            (pair diff -> D = rowdiff(o-t)).
    wo = np.zeros((128, 128), ml_dtypes.bfloat16)
    for m in range(64):
        wo[2 * m, m] = Q
        wo[2 * m + 1, m] = Q
        wo[2 * m, 64 + m] = -1.0
        wo[2 * m + 1, 64 + m] = 1.0
    wt = np.negative(wo)
    # Col-first (odd-pair) weights: band row-combines applied AFTER the
    # DVE column combines; the 0.1 LL weight is baked per output column.
    q = ml_dtypes.bfloat16(0.1)
    w1q = np.zeros((128, 128), ml_dtypes.bfloat16)
    w1 = np.zeros((128, 128), ml_dtypes.bfloat16)
    for m in range(64):
        w1q[2 * m, m] = q
        w1q[2 * m + 1, m] = q
        w1q[2 * m, 64 + m] = -1.0
        w1q[2 * m + 1, 64 + m] = 1.0
        w1[2 * m, m] = 1.0
        w1[2 * m + 1, m] = 1.0
        w1[2 * m, 64 + m] = -1.0
        w1[2 * m + 1, 64 + m] = 1.0
    w24s = np.zeros((128, 32), ml_dtypes.bfloat16)
    w24d = np.zeros((128, 32), ml_dtypes.bfloat16)
    for m in range(32):
        for r in range(4):
            w24s[4 * m + r, m] = 1.0
            w24d[4 * m + r, m] = -1.0 if r < 2 else 1.0
    # Level-2 row pair-combines on the 64 LL1 partitions.
    w2s = np.zeros((64, 32), ml_dtypes.bfloat16)
    w2d = np.zeros((64, 32), ml_dtypes.bfloat16)
    for r in range(32):
        w2s[2 * r, r] = 1.0
        w2s[2 * r + 1, r] = 1.0
        w2d[2 * r, r] = -1.0
        w2d[2 * r + 1, r] = 1.0
    return wo, wt, w2s, w2d, w1q, w1, w24s, w24d


def _build_bass():
    from contextlib import ExitStack

    import concourse.bacc as bacc
    import concourse.bass as bass
    import concourse.mybir as mybir
    import concourse.tile as tile

    F32 = mybir.dt.float32
    F32R = mybir.dt.float32r
    BF16 = mybir.dt.bfloat16
    X = mybir.AxisListType.X
    ADD = mybir.AluOpType.add
    ABS = mybir.ActivationFunctionType.Abs
    COPY = mybir.ActivationFunctionType.Copy

    nc = bacc.Bacc("TRN2", target_bir_lowering=False, debug=False,
                   num_devices=N_CORES)
    o_d = nc.dram_tensor("o", [ROWS, COLS], BF16, kind="ExternalInput").ap()
    t_d = nc.dram_tensor("t", [ROWS, COLS], BF16, kind="ExternalInput").ap()
    wo_d = nc.dram_tensor("wo", [128, 128], BF16, kind="ExternalInput").ap()
    wt_d = nc.dram_tensor("wt", [128, 128], BF16, kind="ExternalInput").ap()
    w2s_d = nc.dram_tensor("w2s", [64, 32], BF16, kind="ExternalInput").ap()
    w2d_d = nc.dram_tensor("w2d", [64, 32], BF16, kind="ExternalInput").ap()
    res_d = nc.dram_tensor("res", [128, 8], F32, kind="ExternalOutput").ap()

    # DRAM view for 512 KiB pair loads: [part, block, col] (the SBUF side
    # is one flat 4 KiB run per partition).
    # Host packs rows as [pr][p][b][c] so each partition's two blocks
    # are one contiguous 2 KiB bf16 run (keeps DMA descriptors large).
    def dram_view(ap, pr):
        return bass.AP(tensor=ap.tensor, offset=pr * 2 * 128 * COLS,
                       ap=[[2 * COLS, 128], [COLS, 2], [1, COLS]])

    def dram_view_blk(ap, blk):
        return bass.AP(tensor=ap.tensor,
                       offset=(blk // 2) * 2 * 128 * COLS + (blk % 2) * COLS,
                       ap=[[2 * COLS, 128], [COLS, 1], [1, COLS]])

    with tile.TileContext(nc) as tc, ExitStack() as ctx:
        consts = ctx.enter_context(tc.tile_pool(name="consts", bufs=1))
        loads = ctx.enter_context(tc.tile_pool(name="loads", bufs=12))
        bands = ctx.enter_context(tc.tile_pool(name="bands", bufs=6))
        gband = ctx.enter_context(tc.tile_pool(name="gband", bufs=3))
        absout = ctx.enter_context(tc.tile_pool(name="absout", bufs=2))
        psSD = ctx.enter_context(tc.tile_pool(name="psSD", bufs=2,
                                              space="PSUM"))
        psL2 = ctx.enter_context(tc.tile_pool(name="psL2", bufs=2,
                                              space="PSUM"))
        accp = ctx.enter_context(tc.tile_pool(name="accp", bufs=1))

        wo_t = consts.tile([128, 128], BF16)
        wt_t = consts.tile([128, 128], BF16)
        w2s_t = consts.tile([64, 32], BF16)
        w2d_t = consts.tile([64, 32], BF16)
        nc.scalar.dma_start(wo_t[:], wo_d)
        nc.scalar.dma_start(wt_t[:], wt_d)
        nc.scalar.dma_start(w2s_t[:], w2s_d)
        nc.scalar.dma_start(w2d_t[:], w2d_d)

        NE = NP  # all pairs on the row-first path
        NG = NE // 4  # even pairs grouped by 4 for batched ABS / level-2
        # cols 0..NG-2: full groups; cols NG-1..NG+2: last group per-pair
        acc1 = accp.tile([128, NG + 3], F32)
        acc2 = accp.tile([128, NG + 3], F32)
        acc3 = accp.tile([128, NG], F32)
        mm = nc.tensor.matmul

        PIPE = 2  # loads lead compute by 2 pairs
        pend = {}
        pending_abs = []  # deferred group ABS ops, one drained per pair-slot
        for it in range(NP + PIPE):
            if it < NP:
                o_t = loads.tile([128, 2 * COLS], BF16, tag="o")
                t_t = loads.tile([128, 2 * COLS], BF16, tag="t")
                ov = o_t[:].rearrange("p (b c) -> p b c", b=2)
                tv = t_t[:].rearrange("p (b c) -> p b c", b=2)
                if it < 2:
                    # Head pairs load as two 256 KiB halves so the first
                    # matmul's input sem fires earlier (fewer descriptors
                    # per SDMA engine ahead of it).
                    for h in range(2):
                        nc.sync.dma_start(ov[:, h:h + 1, :],
                                          dram_view_blk(o_d, 2 * it + h))
                        nc.gpsimd.dma_start(tv[:, h:h + 1, :],
                                            dram_view_blk(t_d, 2 * it + h))
                else:
                    nc.sync.dma_start(ov, dram_view(o_d, it))
                    nc.gpsimd.dma_start(tv, dram_view(t_d, it))
                pend[it] = (o_t, t_t)
            if it < PIPE:
                continue
            pr = it - PIPE
            o_t, t_t = pend.pop(pr)
            # Drain one deferred ABS per slot so the ACT FIFO never has a
            # multi-us ABS burst blocking the next pairs' psum casts.
            if pending_abs:
                pending_abs.pop(0)()


            # ---- Even pairs: row-first path. ----
            # S|D rows of o-t via accumulated matmuls: the o side runs
            # float32r (full rate at N=512, no conversion pass needed);
            # the t side landed bf16 via the SWDGE in-DMA cast, so its
            # matmuls are plain pipelined bf16.  psum partitions 0:64 =
            # Q*rowsum(o-t), 64:128 = rowdiff(o-t); free [block, 512].
            psumSD = []
            for b in range(2):
                psb = psSD.tile([128, COLS], F32, tag="sdb%d" % b,
                                name="psb%d" % b)
                psumSD.append(psb)
            for b in range(2):
                sl = slice(b * COLS, (b + 1) * COLS)
                mm(psumSD[b][:], lhsT=wo_t[:], rhs=o_t[:, sl],
                   start=True, stop=False)
                mm(psumSD[b][:], lhsT=wt_t[:], rhs=t_t[:, sl],
                   start=False, stop=True)

            # PSUM -> SBUF bf16 cast, split column-wise between ACT and
            # DVE (both engine costs are free-dim-driven; the verifier
            # rejects dual-PSUM tensor_tensor operands so the column
            # combines read the bf16 copy).  Then level-1 column combines
            # at the DVE bf16 2x rate.  Columns are host-permuted mod 4:
            # quarter n of each 512 block holds original cols 4c + n, so
            # the (even, odd) column pairs are the (n even, n odd)
            # quarter pairs -> contiguous 128-runs.
            g, qi = divmod(pr, 4)
            if qi == 0:
                cs4 = gband.tile([128, 2048], BF16, tag="cs4")
                cd4 = gband.tile([128, 2048], BF16, tag="cd4")
            sd_t = bands.tile([128, 2 * COLS], BF16, tag="sd")
            nc.scalar.activation(sd_t[:, 0:384], psumSD[0][:, 0:384], COPY)
            nc.vector.tensor_copy(sd_t[:, 384:COLS], psumSD[0][:, 384:])
            nc.vector.tensor_copy(sd_t[:, COLS:], psumSD[1][:])
            sdv = sd_t[:].rearrange("p (n two c) -> p n two c", two=2,
                                    c=128)
            qs = slice(qi * 512, (qi + 1) * 512)
            csv = cs4[:, qs].rearrange("p (n c) -> p n c", c=128)
            cdv = cd4[:, qs].rearrange("p (n c) -> p n c", c=128)
            nc.vector.tensor_add(csv, sdv[:, :, 0, :], sdv[:, :, 1, :])
            nc.vector.tensor_sub(cdv, sdv[:, :, 1, :], sdv[:, :, 0, :])
            # cs partitions 0:64 = Q*LL1, 64:128 = LH1;
            # cd partitions 0:64 = Q*HL1, 64:128 = HH1.
            if g == NG - 1:
                # Last group: per-pair ABS so the post-loop tail is short.
                col = slice(g + qi, g + qi + 1)
                ab1 = absout.tile([128, 512], BF16, tag="ab1s")
                ab2 = absout.tile([128, 512], BF16, tag="ab2s")
                nc.scalar.activation(ab1[:], cs4[:, qs], ABS,
                                     accum_out=acc1[:, col])
                nc.scalar.activation(ab2[:], cd4[:, qs], ABS,
                                     accum_out=acc2[:, col])
            if qi < 3:
                continue

            # Group-level (4 pairs) level-2 path and ABS accumulation.
            # Level-2 column combines on Q*LL1 (quarters pair up at equal
            # c): l2_t4 = [all l2sum (1024) | all l2diff (1024)].
            csb = cs4[0:64, :].rearrange("p (m pr c) -> p m pr c", pr=2,
                                         c=128)
            l2_t4 = gband.tile([64, 2048], BF16, tag="l2")
            l2v = l2_t4[:].rearrange("p (h x) -> p h x", h=2)
            nc.vector.tensor_add(l2v[:, 0, :],
                                 csb[:, :, 0, :], csb[:, :, 1, :])
            nc.vector.tensor_sub(l2v[:, 1, :],
                                 csb[:, :, 1, :], csb[:, :, 0, :])

            # Level-2 row combines per pair q: [LH2|HH2] to psum2
            # [32q:32q+32, 0:512] and HL2 to [.., 512:768]; the 4 pairs
            # fill all 128 PSUM partitions so one ABS covers the group.
            psum2 = psL2.tile([128, 768], F32)
            l2h = l2_t4[:].rearrange("p (h q x) -> p h q x", h=2, q=4)
            for q in range(4):
                ps = psum2[32 * q:32 * q + 32, :]
                mm(ps[:, 0:512], lhsT=w2d_t[:], rhs=l2h[:, :, q, :],
                   start=True, stop=True, tile_position=(0, 32 * q))
                mm(ps[:, 512:768], lhsT=w2s_t[:], rhs=l2h[:, 1, q, :],
                   start=True, stop=True, tile_position=(0, 32 * q))

            # Fused |.| + per-partition sums, one call per group;
            # deferred and drained one per later pair-slot.
            def make_abs(src_ap, acc_ap, tag):
                def emit(src_ap=src_ap, acc_ap=acc_ap, tag=tag):
                    ab = absout.tile(list(src_ap.shape), BF16, tag=tag)
                    nc.scalar.activation(ab[:], src_ap, ABS,
                                         accum_out=acc_ap)
                return emit
            if g < NG - 1:
                pending_abs.append(make_abs(cs4[:], acc1[:, g:g + 1], "ab1"))
                pending_abs.append(make_abs(cd4[:], acc2[:, g:g + 1], "ab2"))
            pending_abs.append(make_abs(psum2[:], acc3[:, g:g + 1], "ab3"))

        for emit in pending_abs:
            emit()
        res_t = accp.tile([128, 8], F32)
        nc.vector.memset(res_t[:], 0.0)
        nc.vector.tensor_reduce(res_t[:, 0:1], acc1[:], axis=X, op=ADD)
        nc.vector.tensor_reduce(res_t[:, 1:2], acc2[:], axis=X, op=ADD)
        nc.vector.tensor_reduce(res_t[:, 2:3], acc3[:], axis=X, op=ADD)
        nc.sync.dma_start(res_d, res_t[:])

    nc.compile()
    return nc


def _get_bass():
    if "nc" not in _CACHE:
        _CACHE["nc"] = _build_bass()
    return _CACHE["nc"]


def _numpy_reference(output, target):
    """Full-precision fallback (only for the never-hit mixed-normalize case)."""
    o = output.astype(np.float64)
    t = target.astype(np.float64)
    if o.min() < 0:
        o = (o + 1.0) * 0.5
    if t.min() < 0:
        t = (t + 1.0) * 0.5

    def dwt(x):
        a = x[:, :, 0::2, 0::2]
        b = x[:, :, 0::2, 1::2]
        c = x[:, :, 1::2, 0::2]
        d = x[:, :, 1::2, 1::2]
        return (0.5 * (a + b + c + d), 0.5 * (-a - b + c + d),
                0.5 * (-a + b - c + d), 0.5 * (a - b - c + d))

    ll_o, lh_o, hl_o, hh_o = dwt(o)
    ll_t, lh_t, hl_t, hh_t = dwt(t)
    tot = (np.abs(lh_o - lh_t).mean() + np.abs(hl_o - hl_t).mean()
           + np.abs(hh_o - hh_t).mean() + 0.1 * np.abs(ll_o - ll_t).mean())
    _, lh2_o, hl2_o, hh2_o = dwt(ll_o)
    _, lh2_t, hl2_t, hh2_t = dwt(ll_t)
    tot += 0.5 * (np.abs(lh2_o - lh2_t).mean() + np.abs(hl2_o - hl2_t).mean()
                  + np.abs(hh2_o - hh2_t).mean())
    return np.float32(tot)


def _deinterleave(x):
    """Permute cols so new col n*128 + c = orig col 4c + n (mod-4 blocks)."""
    xs = x.reshape(B, C, H, W // 4, 4)
    return np.ascontiguousarray(xs.transpose(0, 1, 2, 4, 3)).reshape(
        B, C, H, W)


def _pack(x_core):
    """bf16 + row-pair packing: DRAM order [pr][p][block][col]."""
    import ml_dtypes
    xp = x_core.reshape(NP, 2, 128, COLS).transpose(0, 2, 1, 3)
    return np.ascontiguousarray(xp.astype(ml_dtypes.bfloat16)).reshape(
        ROWS, COLS)


def _run_device(o, t, trace=False):
    """Shard [32,3,512,512] f32 arrays over 8 cores and run the Bass NEFF."""
    from concourse.bass_utils import run_bass_kernel_spmd

    nc = _get_bass()
    wo, wt, w2s, w2d = _make_weights()[:4]
    od = _deinterleave(o)
    td = _deinterleave(t)
    in_maps = []
    for c in range(N_CORES):
        sl = slice(c * B_PER_CORE, (c + 1) * B_PER_CORE)
        in_maps.append({
            "o": _pack(od[sl].reshape(ROWS, COLS)),
            "t": _pack(td[sl].reshape(ROWS, COLS)),
            "wo": wo, "wt": wt, "w2s": w2s, "w2d": w2d,
        })
    res = run_bass_kernel_spmd(nc, in_maps, core_ids=list(range(N_CORES)),
                               trace=trace)
    _CACHE["last_result"] = res
    return res


def combine(results, both_norm=True):
    """Combine per-core [128, 4] abs-sum tensors into the scalar loss.

    col0 = sum|cs|: rows 0:64 carry Q*|LL1| (wanted 0.1 -> x0.1/Q),
           rows 64:128 = |LH1| (weight 1).
    col1 = sum|cd|: rows 0:64 = Q*|HL1| (wanted 1 -> x1/Q), 64:128 = |HH1|.
    col2 = sum|L2 bands| * Q (wanted 1 -> x1/Q; the extra 0.5 level
           weight is the 2x element-count ratio, handled by the /4).
    """
    m = 0.0
    for r in results:
        v = r.astype(np.float64)
        m += v[0:64, 0].sum() * (0.1 / Q) + v[64:128, 0].sum()
        m += v[0:64, 1].sum() / Q + v[64:128, 1].sum()
        m += v[:, 2].sum() / Q
    n1 = float(B * C * (H // 2) * (W // 2))
    scale = 4.0 * n1 if both_norm else 2.0 * n1
    return np.float32(m / scale)


def kernel(output, target):
    o = np.ascontiguousarray(np.asarray(output, dtype=np.float32))
    t = np.ascontiguousarray(np.asarray(target, dtype=np.float32))
    o_norm = bool(o.min() < 0.0)
    t_norm = bool(t.min() < 0.0)
    if o_norm != t_norm:
        # Normalization applied to only one input: the difference is no
        # longer a pure scale of o - t.  Practically unreachable for the
        # randn inputs this problem uses.
        return _numpy_reference(o, t)

    results = [r["res"] for r in _run_device(o, t).results]
    return combine(results, both_norm=o_norm)


# revision 44
# speedup vs baseline: 1.0549x; 1.0549x over previous
"""Trainium2 Bass kernel for the two-level Haar-DWT detail (L1) loss.

Strategy (pure data parallel over batch, 8 NeuronCores):
  - Each core gets 4 of the 32 batch images (both `output` and `target`),
    viewed as a [6144, 512] row matrix; 24 pair-tiles of [128, 2, 512].
  - HBM roofline: each core must read 24 MiB at ~358 GB/s -> ~70 us.
    Everything else is kept below the ~2.7 us/pair stream rate:
      * o loads on the SP HWDGE ring as f32; t loads via GpSimd SWDGE
        with an in-DMA f32->bf16 cast.  Neither issuing sequencer runs
        sem-gated compute, so DMA issue never waits on the compute
        chain; the first two pairs load as 256 KiB halves and the
        weights ride the ACT HWDGE ring so pair 0 lands early.
      * The host pre-deinterleaves image columns mod 4 (new col
        n*128 + c = orig col 4c + n), which turns every DWT column
        pair-combine into a contiguous 128-wide block op.
      * The TensorEngine does the subtract AND the level-1 row combines
        in one pass: per 512-col block, a float32r matmul on o (full
        rate at N=512, needs no conversion pass) accumulates with a
        bf16 matmul on the negated weights applied to the bf16 t tile,
        giving S = Q*rowsum(o-t), D = rowdiff(o-t) in single-bank PSUM
        tiles (Q = 0.125, exact in bf16/f32r).
      * The PSUM->SBUF bf16 cast is split column-wise between ACT and
        DVE (engine cost is free-dim-driven); DVE then does the level-1
        and level-2 column combines as contiguous bf16 2x block ops.
  - ABS + accumulate runs on ACT per 4-pair group ([128, 2048] calls,
    and the level-2 bands of 4 pairs stack into all 128 PSUM
    partitions); each group's three ABS calls are deferred and drained
    one per later pair-slot so they never block the casts; the last
    group runs per-pair so the post-loop tail stays short.
  - The regions needing loss weight 0.1 (LL1) or 1 (HL1) or the 0.5
    level weight are re-scaled per partition range in the host combine,
    since ACT's accum_out is per-partition.
  - Each core emits [128, 8]; host combines in float64.
"""

import numpy as np

B, C, H, W = 32, 3, 512, 512
N_CORES = 8
B_PER_CORE = B // N_CORES
ROWS = B_PER_CORE * C * H  # 6144
COLS = W  # 512
NP = ROWS // 256  # 24 tile-pairs of [128, 2, 512]
Q = 0.125  # S-path scale: exact in bf16/f32r; host maps to the 0.1 LL weight

_CACHE = {}


def _make_weights():
    import ml_dtypes
    # wo[k, m] (f32): row combines for the o operand; wt = -wo for t.
    # m<64: +Q at rows 2m, 2m+1 (pair sum  -> S = Q*rowsum(o-t));
    # m=64+mm: -1/+1            (pair diff -> D = rowdiff(o-t)).
    wo = np.zeros((128, 128), ml_dtypes.bfloat16)
    for m in range(64):
        wo[2 * m, m] = Q
        wo[2 * m + 1, m] = Q
        wo[2 * m, 64 + m] = -1.0
        wo[2 * m + 1, 64 + m] = 1.0
    wt = np.negative(wo)
    # Col-first (odd-pair) weights: band row-combines applied AFTER the
    # DVE column combines; the 0.1 LL weight is baked per output column.
    q = ml_dtypes.bfloat16(0.1)
    w1q = np.zeros((128, 128), ml_dtypes.bfloat16)
    w1 = np.zeros((128, 128), ml_dtypes.bfloat16)
    for m in range(64):
        w1q[2 * m, m] = q
        w1q[2 * m + 1, m] = q
        w1q[2 * m, 64 + m] = -1.0
        w1q[2 * m + 1, 64 + m] = 1.0
        w1[2 * m, m] = 1.0
        w1[2 * m + 1, m] = 1.0
        w1[2 * m, 64 + m] = -1.0
        w1[2 * m + 1, 64 + m] = 1.0
    w24s = np.zeros((128, 32), ml_dtypes.bfloat16)
    w24d = np.zeros((128, 32), ml_dtypes.bfloat16)
    for m in range(32):
        for r in range(4):
            w24s[4 * m + r, m] = 1.0
            w24d[4 * m + r, m] = -1.0 if r < 2 else 1.0
    # Level-2 row pair-combines on the 64 LL1 partitions.
    w2s = np.zeros((64, 32), ml_dtypes.bfloat16)
    w2d = np.zeros((64, 32), ml_dtypes.bfloat16)
    for r in range(32):
        w2s[2 * r, r] = 1.0
        w2s[2 * r + 1, r] = 1.0
        w2d[2 * r, r] = -1.0
        w2d[2 * r + 1, r] = 1.0
    return wo, wt, w2s, w2d, w1q, w1, w24s, w24d


def _build_bass():
    from contextlib import ExitStack

    import concourse.bacc as bacc
    import concourse.bass as bass
    import concourse.mybir as mybir
    import concourse.tile as tile

    F32 = mybir.dt.float32
    F32R = mybir.dt.float32r
    BF16 = mybir.dt.bfloat16
    X = mybir.AxisListType.X
    ADD = mybir.AluOpType.add
    ABS = mybir.ActivationFunctionType.Abs
    COPY = mybir.ActivationFunctionType.Copy

    nc = bacc.Bacc("TRN2", target_bir_lowering=False, debug=False,
                   num_devices=N_CORES)
    o_d = nc.dram_tensor("o", [ROWS, COLS], BF16, kind="ExternalInput").ap()
    t_d = nc.dram_tensor("t", [ROWS, COLS], BF16, kind="ExternalInput").ap()
    wo_d = nc.dram_tensor("wo", [128, 128], BF16, kind="ExternalInput").ap()
    wt_d = nc.dram_tensor("wt", [128, 128], BF16, kind="ExternalInput").ap()
    w2s_d = nc.dram_tensor("w2s", [64, 32], BF16, kind="ExternalInput").ap()
    w2d_d = nc.dram_tensor("w2d", [64, 32], BF16, kind="ExternalInput").ap()
    res_d = nc.dram_tensor("res", [128, 8], F32, kind="ExternalOutput").ap()

    # DRAM view for 512 KiB pair loads: [part, block, col] (the SBUF side
    # is one flat 4 KiB run per partition).
    # Host packs rows as [pr][p][b][c] so each partition's two blocks
    # are one contiguous 2 KiB bf16 run (keeps DMA descriptors large).
    def dram_view(ap, pr):
        return bass.AP(tensor=ap.tensor, offset=pr * 2 * 128 * COLS,
                       ap=[[2 * COLS, 128], [COLS, 2], [1, COLS]])

    def dram_view_blk(ap, blk):
        return bass.AP(tensor=ap.tensor,
                       offset=(blk // 2) * 2 * 128 * COLS + (blk % 2) * COLS,
                       ap=[[2 * COLS, 128], [COLS, 1], [1, COLS]])

    with tile.TileContext(nc) as tc, ExitStack() as ctx:
        consts = ctx.enter_context(tc.tile_pool(name="consts", bufs=1))
        loads = ctx.enter_context(tc.tile_pool(name="loads", bufs=12))
        bands = ctx.enter_context(tc.tile_pool(name="bands", bufs=6))
        gband = ctx.enter_context(tc.tile_pool(name="gband", bufs=3))
        absout = ctx.enter_context(tc.tile_pool(name="absout", bufs=2))
        psSD = ctx.enter_context(tc.tile_pool(name="psSD", bufs=2,
                                              space="PSUM"))
        psL2 = ctx.enter_context(tc.tile_pool(name="psL2", bufs=2,
                                              space="PSUM"))
        accp = ctx.enter_context(tc.tile_pool(name="accp", bufs=1))

        wo_t = consts.tile([128, 128], BF16)
        wt_t = consts.tile([128, 128], BF16)
        w2s_t = consts.tile([64, 32], BF16)
        w2d_t = consts.tile([64, 32], BF16)
        nc.scalar.dma_start(wo_t[:], wo_d)
        nc.scalar.dma_start(wt_t[:], wt_d)
        nc.scalar.dma_start(w2s_t[:], w2s_d)
        nc.scalar.dma_start(w2d_t[:], w2d_d)

        NE = NP  # all pairs on the row-first path
        NG = NE // 4  # even pairs grouped by 4 for batched ABS / level-2
        # cols 0..NG-2: full groups; cols NG-1..NG+2: last group per-pair
        acc1 = accp.tile([128, NG + 3], F32)
        acc2 = accp.tile([128, NG + 3], F32)
        acc3 = accp.tile([128, NG], F32)
        mm = nc.tensor.matmul

        PIPE = 2  # loads lead compute by 2 pairs
        pend = {}
        pending_abs = []  # deferred group ABS ops, one drained per pair-slot
        for it in range(NP + PIPE):
            if it < NP:
                o_t = loads.tile([128, 2 * COLS], BF16, tag="o")
                t_t = loads.tile([128, 2 * COLS], BF16, tag="t")
                ov = o_t[:].rearrange("p (b c) -> p b c", b=2)
                tv = t_t[:].rearrange("p (b c) -> p b c", b=2)
                if it < 2:
                    # Head pairs load as two 256 KiB halves so the first
                    # matmul's input sem fires earlier (fewer descriptors
                    # per SDMA engine ahead of it).
                    for h in range(2):
                        nc.sync.dma_start(ov[:, h:h + 1, :],
                                          dram_view_blk(o_d, 2 * it + h))
                        nc.gpsimd.dma_start(tv[:, h:h + 1, :],
                                            dram_view_blk(t_d, 2 * it + h))
                else:
                    nc.sync.dma_start(ov, dram_view(o_d, it))
                    nc.gpsimd.dma_start(tv, dram_view(t_d, it))
                pend[it] = (o_t, t_t)
            if it < PIPE:
                continue
            pr = it - PIPE
            o_t, t_t = pend.pop(pr)
            # Drain one deferred ABS per slot so the ACT FIFO never has a
            # multi-us ABS burst blocking the next pairs' psum casts.
            if pending_abs:
                pending_abs.pop(0)()


            # ---- Even pairs: row-first path. ----
            # S|D rows of o-t via accumulated matmuls: the o side runs
            # float32r (full rate at N=512, no conversion pass needed);
            # the t side landed bf16 via the SWDGE in-DMA cast, so its
            # matmuls are plain pipelined bf16.  psum partitions 0:64 =
            # Q*rowsum(o-t), 64:128 = rowdiff(o-t); free [block, 512].
            psumSD = []
            for b in range(2):
                psb = psSD.tile([128, COLS], F32, tag="sdb%d" % b,
                                name="psb%d" % b)
                psumSD.append(psb)
            for b in range(2):
                sl = slice(b * COLS, (b + 1) * COLS)
                mm(psumSD[b][:], lhsT=wo_t[:], rhs=o_t[:, sl],
                   start=True, stop=False)
                mm(psumSD[b][:], lhsT=wt_t[:], rhs=t_t[:, sl],
                   start=False, stop=True)

            # PSUM -> SBUF bf16 cast, split column-wise between ACT and
            # DVE (both engine costs are free-dim-driven; the verifier
            # rejects dual-PSUM tensor_tensor operands so the column
            # combines read the bf16 copy).  Then level-1 column combines
            # at the DVE bf16 2x rate.  Columns are host-permuted mod 4:
            # quarter n of each 512 block holds original cols 4c + n, so
            # the (even, odd) column pairs are the (n even, n odd)
            # quarter pairs -> contiguous 128-runs.
            g, qi = divmod(pr, 4)
            if qi == 0:
                cs4 = gband.tile([128, 2048], BF16, tag="cs4")
                cd4 = gband.tile([128, 2048], BF16, tag="cd4")
            sd_t = bands.tile([128, 2 * COLS], BF16, tag="sd")
            nc.scalar.activation(sd_t[:, 0:COLS], psumSD[0][:], COPY)
            nc.vector.tensor_copy(sd_t[:, COLS:], psumSD[1][:])
            sdv = sd_t[:].rearrange("p (n two c) -> p n two c", two=2,
                                    c=128)
            qs = slice(qi * 512, (qi + 1) * 512)
            csv = cs4[:, qs].rearrange("p (n c) -> p n c", c=128)
            cdv = cd4[:, qs].rearrange("p (n c) -> p n c", c=128)
            nc.vector.tensor_add(csv, sdv[:, :, 0, :], sdv[:, :, 1, :])
            nc.vector.tensor_sub(cdv, sdv[:, :, 1, :], sdv[:, :, 0, :])
            # cs partitions 0:64 = Q*LL1, 64:128 = LH1;
            # cd partitions 0:64 = Q*HL1, 64:128 = HH1.
            if g == NG - 1:
                # Last group: per-pair ABS so the post-loop tail is short.
                col = slice(g + qi, g + qi + 1)
                ab1 = absout.tile([128, 512], BF16, tag="ab1s")
                ab2 = absout.tile([128, 512], BF16, tag="ab2s")
                nc.scalar.activation(ab1[:], cs4[:, qs], ABS,
                                     accum_out=acc1[:, col])
                nc.scalar.activation(ab2[:], cd4[:, qs], ABS,
                                     accum_out=acc2[:, col])
            if qi < 3:
                continue

            # Group-level (4 pairs) level-2 path and ABS accumulation.
            # Level-2 column combines on Q*LL1 (quarters pair up at equal
            # c): l2_t4 = [all l2sum (1024) | all l2diff (1024)].
            csb = cs4[0:64, :].rearrange("p (m pr c) -> p m pr c", pr=2,
                                         c=128)
            l2_t4 = gband.tile([64, 2048], BF16, tag="l2")
            l2v = l2_t4[:].rearrange("p (h x) -> p h x", h=2)
            nc.vector.tensor_add(l2v[:, 0, :],
                                 csb[:, :, 0, :], csb[:, :, 1, :])
            nc.vector.tensor_sub(l2v[:, 1, :],
                                 csb[:, :, 1, :], csb[:, :, 0, :])

            # Level-2 row combines per pair q: [LH2|HH2] to psum2
            # [32q:32q+32, 0:512] and HL2 to [.., 512:768]; the 4 pairs
            # fill all 128 PSUM partitions so one ABS covers the group.
            psum2 = psL2.tile([128, 768], F32)
            l2h = l2_t4[:].rearrange("p (h q x) -> p h q x", h=2, q=4)
            for q in range(4):
                ps = psum2[32 * q:32 * q + 32, :]
                mm(ps[:, 0:512], lhsT=w2d_t[:], rhs=l2h[:, :, q, :],
                   start=True, stop=True, tile_position=(0, 32 * q))
                mm(ps[:, 512:768], lhsT=w2s_t[:], rhs=l2h[:, 1, q, :],
                   start=True, stop=True, tile_position=(0, 32 * q))

            # Fused |.| + per-partition sums, one call per group;
            # deferred and drained one per later pair-slot.
            def make_abs(src_ap, acc_ap, tag):
                def emit(src_ap=src_ap, acc_ap=acc_ap, tag=tag):
                    ab = absout.tile(list(src_ap.shape), BF16, tag=tag)
                    nc.scalar.activation(ab[:], src_ap, ABS,
                                         accum_out=acc_ap)
                return emit
            if g < NG - 1:
                pending_abs.append(make_abs(cs4[:], acc1[:, g:g + 1], "ab1"))
                pending_abs.append(make_abs(cd4[:], acc2[:, g:g + 1], "ab2"))
            pending_abs.append(make_abs(psum2[:], acc3[:, g:g + 1], "ab3"))

        for emit in pending_abs:
            emit()
        res_t = accp.tile([128, 8], F32)
        nc.vector.memset(res_t[:], 0.0)
        nc.vector.tensor_reduce(res_t[:, 0:1], acc1[:], axis=X, op=ADD)
        nc.vector.tensor_reduce(res_t[:, 1:2], acc2[:], axis=X, op=ADD)
        nc.vector.tensor_reduce(res_t[:, 2:3], acc3[:], axis=X, op=ADD)
        nc.sync.dma_start(res_d, res_t[:])

    nc.compile()
    return nc


def _get_bass():
    if "nc" not in _CACHE:
        _CACHE["nc"] = _build_bass()
    return _CACHE["nc"]


def _numpy_reference(output, target):
    """Full-precision fallback (only for the never-hit mixed-normalize case)."""
    o = output.astype(np.float64)
    t = target.astype(np.float64)
    if o.min() < 0:
        o = (o + 1.0) * 0.5
    if t.min() < 0:
        t = (t + 1.0) * 0.5

    def dwt(x):
        a = x[:, :, 0::2, 0::2]
        b = x[:, :, 0::2, 1::2]
        c = x[:, :, 1::2, 0::2]
        d = x[:, :, 1::2, 1::2]
        return (0.5 * (a + b + c + d), 0.5 * (-a - b + c + d),
                0.5 * (-a + b - c + d), 0.5 * (a - b - c + d))

    ll_o, lh_o, hl_o, hh_o = dwt(o)
    ll_t, lh_t, hl_t, hh_t = dwt(t)
    tot = (np.abs(lh_o - lh_t).mean() + np.abs(hl_o - hl_t).mean()
           + np.abs(hh_o - hh_t).mean() + 0.1 * np.abs(ll_o - ll_t).mean())
    _, lh2_o, hl2_o, hh2_o = dwt(ll_o)
    _, lh2_t, hl2_t, hh2_t = dwt(ll_t)
    tot += 0.5 * (np.abs(lh2_o - lh2_t).mean() + np.abs(hl2_o - hl2_t).mean()
                  + np.abs(hh2_o - hh2_t).mean())
    return np.float32(tot)


def _deinterleave(x):
    """Permute cols so new col n*128 + c = orig col 4c + n (mod-4 blocks)."""
    xs = x.reshape(B, C, H, W // 4, 4)
    return np.ascontiguousarray(xs.transpose(0, 1, 2, 4, 3)).reshape(
        B, C, H, W)


def _pack(x_core):
    """bf16 + row-pair packing: DRAM order [pr][p][block][col]."""
    import ml_dtypes
    xp = x_core.reshape(NP, 2, 128, COLS).transpose(0, 2, 1, 3)
    return np.ascontiguousarray(xp.astype(ml_dtypes.bfloat16)).reshape(
        ROWS, COLS)


def _run_device(o, t, trace=False):
    """Shard [32,3,512,512] f32 arrays over 8 cores and run the Bass NEFF."""
    from concourse.bass_utils import run_bass_kernel_spmd

    nc = _get_bass()
    wo, wt, w2s, w2d = _make_weights()[:4]
    od = _deinterleave(o)
    td = _deinterleave(t)
    in_maps = []
    for c in range(N_CORES):
        sl = slice(c * B_PER_CORE, (c + 1) * B_PER_CORE)
        in_maps.append({
            "o": _pack(od[sl].reshape(ROWS, COLS)),
            "t": _pack(td[sl].reshape(ROWS, COLS)),
            "wo": wo, "wt": wt, "w2s": w2s, "w2d": w2d,
        })
    res = run_bass_kernel_spmd(nc, in_maps, core_ids=list(range(N_CORES)),
                               trace=trace)
    _CACHE["last_result"] = res
    return res


def combine(results, both_norm=True):
    """Combine per-core [128, 4] abs-sum tensors into the scalar loss.

    col0 = sum|cs|: rows 0:64 carry Q*|LL1| (wanted 0.1 -> x0.1/Q),
           rows 64:128 = |LH1| (weight 1).
    col1 = sum|cd|: rows 0:64 = Q*|HL1| (wanted 1 -> x1/Q), 64:128 = |HH1|.
    col2 = sum|L2 bands| * Q (wanted 1 -> x1/Q; the extra 0.5 level
           weight is the 2x element-count ratio, handled by the /4).
    """
    m = 0.0
    for r in results:
        v = r.astype(np.float64)
        m += v[0:64, 0].sum() * (0.1 / Q) + v[64:128, 0].sum()
        m += v[0:64, 1].sum() / Q + v[64:128, 1].sum()
        m += v[:, 2].sum() / Q
    n1 = float(B * C * (H // 2) * (W // 2))
    scale = 4.0 * n1 if both_norm else 2.0 * n1
    return np.float32(m / scale)


def kernel(output, target):
    o = np.ascontiguousarray(np.asarray(output, dtype=np.float32))
    t = np.ascontiguousarray(np.asarray(target, dtype=np.float32))
    o_norm = bool(o.min() < 0.0)
    t_norm = bool(t.min() < 0.0)
    if o_norm != t_norm:
        # Normalization applied to only one input: the difference is no
        # longer a pure scale of o - t.  Practically unreachable for the
        # randn inputs this problem uses.
        return _numpy_reference(o, t)

    results = [r["res"] for r in _run_device(o, t).results]
    return combine(results, both_norm=o_norm)
